# revision 1
# baseline (speedup 1.0000x reference)
"""Trainium2 Bass kernel for nn_GroupFeatureBuilder (segment_reduce).

Strategy: shard the M=4096 groups across 8 cores (512 groups each).
Replace all gathers with dense matmuls against a host-built multiplicity
matrix C[m, n] = (# occurrences of robot n in group m):

  E      = C @ A            (A = attn_rr, bf16)       -> per-group attn rows
  t1[m]  = <E[m], C[m]>     = sum_{i,j} A[g_i, g_j]
  t2[m]  = <C[m]^2, diagA>  = sum over equal pairs
  t3[m]  = <E[m], mem[m]>   (mem = min(C,1), the unique-membership mask)
  esum[m]= sum_n E[m,n]     = rows.sum
  a_in   = (t1 - t2) / max(256 - sum C^2, 1)
  a_out  = (esum - t3) / (16 * (2048 - sum mem))
  HR     = C @ [h | attn_ro]  -> h_g (cols 0:256), a_obs (rowsum of 256:320)
  h_glob = ones^T @ h / 2048
  ex_dist/ex_clr: host-gathered (512,16) slots, device mean/min reduce.
"""

import numpy as np
import ml_dtypes

import concourse.bass as bass
import concourse.bacc as bacc
import concourse.tile as tile
import concourse.mybir as mybir
from concourse.bass_utils import run_bass_kernel_spmd

BF16 = ml_dtypes.bfloat16

N = 2048       # robots
D = 256        # embed
M = 4096       # groups
K = 16         # group size
NOBS = 64
NCORES = 8
MLOC = M // NCORES     # 512 groups per core
MCH = MLOC // 128      # 4 m-chunks
KCH = N // 128         # 16 contraction chunks
NCH = N // 512         # 4 column chunks of A
HRW = D + NOBS         # 320 columns of the HR rhs
FOUT = 2 * D + 6       # 518 output features

f32 = mybir.dt.float32
bf16 = mybir.dt.bfloat16
OP = mybir.AluOpType
AX = mybir.AxisListType
ACT = mybir.ActivationFunctionType

_NC_CACHE = {}


def _build_nc():
    nc = bacc.Bacc("TRN2", target_bir_lowering=False, debug=False,
                   num_devices=NCORES)

    a_d = nc.declare_dram_parameter("a_bf", [N, N], bf16, isOutput=False)
    hr_d = nc.declare_dram_parameter("hr_bf", [N, HRW], bf16, isOutput=False)
    ct_d = nc.declare_dram_parameter("ct_bf", [N, MLOC], bf16, isOutput=False)
    cm_d = nc.declare_dram_parameter("cm_bf", [MLOC, N], bf16, isOutput=False)
    diag_d = nc.declare_dram_parameter("diag_bf", [1, N], bf16, isOutput=False)
    cg_d = nc.declare_dram_parameter("cg", [MLOC, K], f32, isOutput=False)
    dg_d = nc.declare_dram_parameter("dg", [MLOC, K], f32, isOutput=False)
    out_d = nc.declare_dram_parameter("out", [MLOC, FOUT], f32, isOutput=True)

    with tile.TileContext(nc) as tc:
        with (
            tc.tile_pool(name="res", bufs=1) as res,
            tc.tile_pool(name="apool", bufs=1) as apool,
            tc.tile_pool(name="junk", bufs=3) as junkp,
            tc.tile_pool(name="stats", bufs=1) as statp,
            tc.tile_pool(name="psum_e", bufs=5, space="PSUM") as pe_pool,
            tc.tile_pool(name="psum_hr", bufs=2, space="PSUM") as phr_pool,
            tc.tile_pool(name="psum_g", bufs=1, space="PSUM") as pg_pool,
        ):
            # ---- resident loads ----
            a_t = []
            for k in range(KCH):
                t = apool.tile([128, N], bf16, tag=f"a{k}")
                nc.sync.dma_start(out=t[:], in_=a_d[k * 128:(k + 1) * 128, :])
                a_t.append(t)
            hr_t = []
            for k in range(KCH):
                t = res.tile([128, HRW], bf16, tag=f"hr{k}")
                nc.sync.dma_start(out=t[:], in_=hr_d[k * 128:(k + 1) * 128, :])
                hr_t.append(t)
            ct_t = []
            for k in range(KCH):
                t = res.tile([128, MLOC], bf16, tag=f"ct{k}")
                nc.sync.dma_start(out=t[:], in_=ct_d[k * 128:(k + 1) * 128, :])
                ct_t.append(t)
            cm_t = []
            for m in range(MCH):
                t = res.tile([128, N], bf16, tag=f"cm{m}")
                nc.sync.dma_start(out=t[:], in_=cm_d[m * 128:(m + 1) * 128, :])
                cm_t.append(t)
            cg_t = []
            dg_t = []
            for m in range(MCH):
                t = res.tile([128, K], f32, tag=f"cg{m}")
                nc.sync.dma_start(out=t[:], in_=cg_d[m * 128:(m + 1) * 128, :])
                cg_t.append(t)
                t = res.tile([128, K], f32, tag=f"dg{m}")
                nc.sync.dma_start(out=t[:], in_=dg_d[m * 128:(m + 1) * 128, :])
                dg_t.append(t)

            diag_row = res.tile([1, N], bf16, tag="diag_row")
            nc.sync.dma_start(out=diag_row[:], in_=diag_d[:, :])

            ones_t = res.tile([128, 1], bf16, tag="ones")
            nc.vector.memset(ones_t[:], 1.0)
            ones_row = res.tile([1, 128], bf16, tag="ones_row")
            nc.vector.memset(ones_row[:], 1.0)

            # broadcast diag across partitions via K=1 outer-product matmuls
            diag_b = res.tile([128, N], bf16, tag="diag_b")
            for n in range(NCH):
                pb = pe_pool.tile([128, 512], f32, tag="pe", name=f"pbd{n}")
                nc.tensor.matmul(pb[:], ones_row[:1],
                                 diag_row[:1, n * 512:(n + 1) * 512])
                nc.scalar.activation(diag_b[:, n * 512:(n + 1) * 512], pb[:],
                                     ACT.Copy)

            # ---- h_glob = mean_n h[n, :] via ones^T @ h ----
            pg = pg_pool.tile([1, D], f32)
            for k in range(KCH):
                nc.tensor.matmul(pg[:], ones_t[:], hr_t[k][:, 0:D],
                                 start=(k == 0), stop=(k == KCH - 1))
            hglob_row = res.tile([1, D], bf16, tag="hglob_row")
            nc.scalar.activation(hglob_row[:], pg[:], ACT.Copy, scale=1.0 / N)
            hglob_b = res.tile([128, D], f32, tag="hglob_b")
            pgb = pg_pool.tile([128, D], f32, name="pgb", tag="pg")
            nc.tensor.matmul(pgb[:], ones_row[:1], hglob_row[:1])
            nc.scalar.activation(hglob_b[:], pgb[:], ACT.Copy)

            # ---- per m-chunk ----
            for m in range(MCH):
                ms, me = m * 128, (m + 1) * 128
                out_t = res.tile([128, FOUT], f32, tag=f"out{m}")

                # membership mask + n_uniq, C^2 + sum C^2, t2
                mem = res.tile([128, N], bf16, tag=f"mem{m}")
                nuniq = statp.tile([128, 1], f32, tag=f"nu{m}")
                nc.vector.tensor_scalar(out=mem[:], in0=cm_t[m][:], scalar1=1.0,
                                        scalar2=0.0, op0=OP.min, op1=OP.add,
                                        accum_out=nuniq[:])
                cc = junkp.tile([128, N], bf16, tag="cc")
                sumcc = statp.tile([128, 1], f32, tag=f"scc{m}")
                nc.vector.tensor_mul(cc[:], cm_t[m][:], cm_t[m][:])
                nc.vector.tensor_reduce(sumcc[:], cc[:], AX.X, OP.add)
                jk = junkp.tile([128, N], bf16, tag="jk")
                t2 = statp.tile([128, 1], f32, tag=f"t2{m}")
                nc.vector.tensor_mul(jk[:], cc[:], diag_b[:])
                nc.vector.tensor_reduce(t2[:], jk[:], AX.X, OP.add)

                # HR matmul: h_g + a_obs
                phr = phr_pool.tile([128, HRW], f32)
                for k in range(KCH):
                    nc.tensor.matmul(phr[:], ct_t[k][:, ms:me], hr_t[k][:],
                                     start=(k == 0), stop=(k == KCH - 1))
                nc.scalar.activation(out_t[:, 0:D], phr[:, 0:D], ACT.Copy,
                                     scale=1.0 / K)
                aobs = statp.tile([128, 1], f32, tag=f"ao{m}")
                nc.vector.tensor_reduce(aobs[:], phr[:, D:HRW], AX.X, OP.add)
                nc.vector.tensor_scalar_mul(out_t[:, 515:516], aobs[:],
                                            1.0 / (K * NOBS))

                # E matmul: 4 psum banks accumulate over k
                pe_n = []
                for n in range(NCH):
                    pe_n.append(pe_pool.tile([128, 512], f32, tag="pe",
                                             name=f"pe{m}_{n}"))
                for k in range(KCH):
                    for n in range(NCH):
                        nc.tensor.matmul(pe_n[n][:], ct_t[k][:, ms:me],
                                         a_t[k][:, n * 512:(n + 1) * 512],
                                         start=(k == 0), stop=(k == KCH - 1))

                t1p = statp.tile([128, NCH], f32, tag=f"t1p{m}")
                t3p = statp.tile([128, NCH], f32, tag=f"t3p{m}")
                esp = statp.tile([128, NCH], f32, tag=f"esp{m}")
                for n in range(NCH):
                    j1 = junkp.tile([128, 512], bf16, tag="j1")
                    nc.vector.tensor_mul(j1[:], pe_n[n][:],
                                         cm_t[m][:, n * 512:(n + 1) * 512])
                    nc.vector.tensor_reduce(t1p[:, n:n + 1], j1[:], AX.X,
                                            OP.add)
                    j2 = junkp.tile([128, 512], bf16, tag="j2")
                    nc.vector.tensor_mul(j2[:], pe_n[n][:],
                                         mem[:, n * 512:(n + 1) * 512])
                    nc.vector.tensor_reduce(t3p[:, n:n + 1], j2[:], AX.X,
                                            OP.add)
                    nc.vector.tensor_reduce(esp[:, n:n + 1], pe_n[n][:],
                                            AX.X, OP.add)

                # ---- scalar fixups ----
                t1s = statp.tile([128, 1], f32, tag=f"t1s{m}")
                nc.vector.tensor_reduce(t1s[:], t1p[:], AX.X, OP.add)
                t3s = statp.tile([128, 1], f32, tag=f"t3s{m}")
                nc.vector.tensor_reduce(t3s[:], t3p[:], AX.X, OP.add)
                ess = statp.tile([128, 1], f32, tag=f"ess{m}")
                nc.vector.tensor_reduce(ess[:], esp[:], AX.X, OP.add)

                # a_in = (t1 - t2) / max(K*K - sumcc, 1)
                cnt = statp.tile([128, 1], f32, tag=f"cnt{m}")
                nc.vector.tensor_scalar(out=cnt[:], in0=sumcc[:], scalar1=-1.0,
                                        scalar2=float(K * K), op0=OP.mult,
                                        op1=OP.add)
                cntm = statp.tile([128, 1], f32, tag=f"cntm{m}")
                nc.vector.tensor_scalar_max(cntm[:], cnt[:], 1.0)
                rin = statp.tile([128, 1], f32, tag=f"rin{m}")
                nc.vector.reciprocal(rin[:], cntm[:])
                num_in = statp.tile([128, 1], f32, tag=f"ni{m}")
                nc.vector.tensor_sub(num_in[:], t1s[:], t2[:])
                nc.vector.tensor_mul(out_t[:, 513:514], num_in[:], rin[:])

                # a_out = (esum - t3) / (K * (N - nuniq))
                den = statp.tile([128, 1], f32, tag=f"den{m}")
                nc.vector.tensor_scalar(out=den[:], in0=nuniq[:],
                                        scalar1=-float(K),
                                        scalar2=float(K * N), op0=OP.mult,
                                        op1=OP.add)
                rout = statp.tile([128, 1], f32, tag=f"ro{m}")
                nc.vector.reciprocal(rout[:], den[:])
                num_out = statp.tile([128, 1], f32, tag=f"no{m}")
                nc.vector.tensor_sub(num_out[:], ess[:], t3s[:])
                nc.vector.tensor_mul(out_t[:, 514:515], num_out[:], rout[:])

                # ex_dist (mean), ex_clr (min)
                exd = statp.tile([128, 1], f32, tag=f"exd{m}")
                nc.vector.tensor_reduce(exd[:], dg_t[m][:], AX.X, OP.add)
                nc.vector.tensor_scalar_mul(out_t[:, 516:517], exd[:], 1.0 / K)
                nc.vector.tensor_reduce(out_t[:, 517:518], cg_t[m][:], AX.X,
                                        OP.min)

                # h_glob + size_feat
                nc.scalar.activation(out_t[:, D:2 * D], hglob_b[:], ACT.Copy)
                nc.vector.memset(out_t[:, 512:513], float(K) / 3.0)

                nc.sync.dma_start(out=out_d[m * 128:(m + 1) * 128, :],
                                  in_=out_t[:])
    nc.compile()
    return nc


def _get_nc():
    if "nc" not in _NC_CACHE:
        _NC_CACHE["nc"] = _build_nc()
    return _NC_CACHE["nc"]


def kernel(h, attn_rr, attn_ro, dist_to_goal, clearance, groups):
    h = np.asarray(h, dtype=np.float32)
    attn_rr = np.asarray(attn_rr, dtype=np.float32)
    attn_ro = np.asarray(attn_ro, dtype=np.float32)
    dist_to_goal = np.asarray(dist_to_goal, dtype=np.float32)
    clearance = np.asarray(clearance, dtype=np.float32)
    groups = np.asarray(groups)

    a_bf = np.ascontiguousarray(attn_rr.astype(BF16))
    hr_bf = np.ascontiguousarray(
        np.concatenate([h, attn_ro], axis=1).astype(BF16))
    diag_bf = np.ascontiguousarray(np.diagonal(attn_rr)[None, :].astype(BF16))

    in_maps = []
    for s in range(NCORES):
        gs = groups[s * MLOC:(s + 1) * MLOC]
        C = np.zeros((MLOC, N), dtype=np.float32)
        np.add.at(C, (np.arange(MLOC)[:, None], gs), 1.0)
        in_maps.append({
            "a_bf": a_bf,
            "hr_bf": hr_bf,
            "ct_bf": np.ascontiguousarray(C.T.astype(BF16)),
            "cm_bf": np.ascontiguousarray(C.astype(BF16)),
            "diag_bf": diag_bf,
            "cg": np.ascontiguousarray(clearance[gs].astype(np.float32)),
            "dg": np.ascontiguousarray(dist_to_goal[gs].astype(np.float32)),
        })

    nc = _get_nc()
    _NC_CACHE["last_in_maps"] = in_maps
    res = run_bass_kernel_spmd(nc, in_maps, list(range(NCORES)))
    return np.concatenate([res.results[s]["out"] for s in range(NCORES)],
                          axis=0)



# revision 10
# speedup vs baseline: 1.7204x; 1.7204x over previous
"""Trainium2 Bass kernel for nn_GroupFeatureBuilder (segment_reduce).

Strategy: shard the M=4096 groups across 8 cores (512 groups each).
Replace all gathers with dense matmuls against a host-built multiplicity
matrix C[m, n] = (# occurrences of robot n in group m):

  E      = C @ A            (A = attn_rr, fp8 DoubleRow)   -> per-group rows
  t1[m]  = <E[m], C[m]>     = sum_{i,j} A[g_i, g_j]  (fused DVE rowdot)
  t3     ~= t1              (duplicate correction dropped; |err| ~1e-4 rel)
  esum[m]= C @ rowsumA      (extra bf16 column in the HR matmul)
  a_in   = (t1 - t2) * invcnt     t2, invcnt, invden host-precomputed
  a_out  = (esum - t1) * invden   from group indices / tiny vector gathers
  HR     = C @ [h | attn_ro | rowsumA/64]  (bf16)  -> h_g, a_obs, esum
  h_glob = ones^T @ h / 2048 (device)
  ex_dist/ex_clr/t2: host-gathered (512,16) f32 slots, device reduce.
"""

import os

import numpy as np
import ml_dtypes

_EMODE = os.environ.get("EMODE", "dr")

import concourse.bass as bass
import concourse.bacc as bacc
import concourse.tile as tile
import concourse.mybir as mybir
from concourse.bass_utils import run_bass_kernel_spmd

BF16 = ml_dtypes.bfloat16
F8 = ml_dtypes.float8_e4m3

N = 2048       # robots
D = 256        # embed
M = 4096       # groups
K = 16         # group size
NOBS = 64
NCORES = 8
MLOC = M // NCORES     # 512 groups per core
MCH = MLOC // 128      # 4 m-chunks
JCH = N // 256         # 8 double-row contraction blocks
KCH = N // 128         # 16 normal contraction blocks
NCH = N // 512         # 4 column chunks of A
HRW = D + NOBS + 1     # 321 columns of the HR rhs (h | ro | rowsumA/64)
SLW = 52               # slot cols: dg16 | cg16 | ds2_16 | invcnt | invden | pad
FOUT = 2 * D + 6       # 518 output features

f32 = mybir.dt.float32
bf16 = mybir.dt.bfloat16
f8 = mybir.dt.float8e4
OP = mybir.AluOpType
AX = mybir.AxisListType
ACT = mybir.ActivationFunctionType
DR = mybir.MatmulPerfMode.DoubleRow

_NC_CACHE = {}


def _build_nc():
    nc = bacc.Bacc("TRN2", target_bir_lowering=False, debug=False,
                   num_devices=NCORES)

    edt = bf16 if _EMODE == "bf16" else f8
    a8_d = nc.declare_dram_parameter("a8", [JCH, 128, 2, N], edt,
                                     isOutput=False)
    c8_d = nc.declare_dram_parameter("c8", [JCH, 128, 2, MLOC], edt,
                                     isOutput=False)
    ctb_d = nc.declare_dram_parameter("ctb", [KCH, 128, MLOC], bf16,
                                      isOutput=False)
    cm_d = nc.declare_dram_parameter("cm", [MCH, 128, N], bf16,
                                     isOutput=False)
    hr_d = nc.declare_dram_parameter("hr", [KCH, 128, HRW], bf16,
                                     isOutput=False)
    sl_d = nc.declare_dram_parameter("sl", [MCH, 128, SLW], f32,
                                     isOutput=False)
    out_d = nc.declare_dram_parameter("out", [MCH, 128, FOUT], f32,
                                      isOutput=True)

    with tile.TileContext(nc) as tc:
        with (
            tc.tile_pool(name="res", bufs=1) as res,
            tc.tile_pool(name="junk", bufs=2) as junkp,
            tc.tile_pool(name="stats", bufs=1) as statp,
            tc.tile_pool(name="psum_e", bufs=1, space="PSUM") as pe_pool,
            tc.tile_pool(name="psum_hr", bufs=2, space="PSUM") as phr_pool,
            tc.tile_pool(name="psum_g", bufs=1, space="PSUM") as pg_pool,
        ):
            # ---- resident loads, in consumption order ----
            sl_t = []
            for m in range(MCH):
                t = res.tile([128, SLW], f32, tag=f"sl{m}")
                nc.sync.dma_start(out=t[:], in_=sl_d[m])
                sl_t.append(t)
            ctb_t = []
            hr_t = []
            for k in range(KCH):
                t = res.tile([128, MLOC], bf16, tag=f"ctb{k}")
                nc.sync.dma_start(out=t[:], in_=ctb_d[k])
                ctb_t.append(t)
                t = res.tile([128, HRW], bf16, tag=f"hr{k}")
                nc.sync.dma_start(out=t[:], in_=hr_d[k])
                hr_t.append(t)
            c8_t = []
            a8_t = []
            for j in range(JCH):
                t = res.tile([128, 2, MLOC], edt, tag=f"c8{j}")
                nc.sync.dma_start(out=t[:], in_=c8_d[j])
                c8_t.append(t)
                t = res.tile([128, 2, N], edt, tag=f"a8{j}")
                nc.sync.dma_start(out=t[0:64], in_=a8_d[j, 0:64])
                nc.sync.dma_start(out=t[64:128], in_=a8_d[j, 64:128])
                a8_t.append(t)
            cm_t = []
            for m in range(MCH):
                t = res.tile([128, N], bf16, tag=f"cm{m}")
                nc.sync.dma_start(out=t[:], in_=cm_d[m])
                cm_t.append(t)

            ones_t = res.tile([128, 1], bf16, tag="ones")
            nc.vector.memset(ones_t[:], 1.0)
            ones_row = res.tile([1, 128], bf16, tag="ones_row")
            nc.vector.memset(ones_row[:], 1.0)

            out_t = []
            es_t = []
            for m in range(MCH):
                t = res.tile([128, FOUT], f32, tag=f"out{m}")
                out_t.append(t)
                nc.vector.memset(t[:, 512:513], float(K) / 3.0)
                es_t.append(statp.tile([128, 1], f32, tag=f"es{m}",
                                       name=f"es{m}"))

            # ---- HR matmuls: h_g, a_obs, esum ----
            for m in range(MCH):
                ms, me = m * 128, (m + 1) * 128
                phr = phr_pool.tile([128, HRW], f32, tag="phr",
                                    name=f"phr{m}")
                for k in range(KCH):
                    nc.tensor.matmul(phr[:], ctb_t[k][:, ms:me], hr_t[k][:],
                                     start=(k == 0), stop=(k == KCH - 1))
                nc.scalar.activation(out_t[m][:, 0:D], phr[:, 0:D], ACT.Copy,
                                     scale=1.0 / K)
                aob = statp.tile([128, 1], f32, tag=f"ao{m}")
                nc.vector.tensor_reduce(aob[:], phr[:, D:D + NOBS], AX.X,
                                        OP.add)
                nc.vector.tensor_scalar_mul(out_t[m][:, 515:516], aob[:],
                                            1.0 / (K * NOBS))
                nc.scalar.activation(es_t[m][:], phr[:, 320:321], ACT.Copy,
                                     scale=64.0)

            # ---- h_glob = mean_n h[n, :] via ones^T @ h, then broadcast ----
            pg = pg_pool.tile([1, D], f32, tag="pg")
            for k in range(KCH):
                nc.tensor.matmul(pg[:], ones_t[:], hr_t[k][:, 0:D],
                                 start=(k == 0), stop=(k == KCH - 1))
            hglob_row = res.tile([1, D], bf16, tag="hglob_row")
            nc.scalar.activation(hglob_row[:], pg[:], ACT.Copy, scale=1.0 / N)
            pgb = pg_pool.tile([128, D], f32, tag="pgb")
            nc.tensor.matmul(pgb[:], ones_row[:1], hglob_row[:1])

            # ---- E matmuls (fp8 DoubleRow) + per-chunk stats ----
            for m in range(MCH):
                ms, me = m * 128, (m + 1) * 128
                pe_n = []
                for n in range(NCH):
                    pe_n.append(pe_pool.tile([128, 512], f32, tag=f"pe{n}",
                                             name=f"pe{m}_{n}"))
                if _EMODE == "dr":
                    for j in range(JCH):
                        for n in range(NCH):
                            nc.tensor.matmul(
                                pe_n[n][:], c8_t[j][:, :, ms:me],
                                a8_t[j][:, :, n * 512:(n + 1) * 512],
                                start=(j == 0), stop=(j == JCH - 1),
                                perf_mode=DR)
                else:
                    for j in range(JCH):
                        for s in range(2):
                            for n in range(NCH):
                                nc.tensor.matmul(
                                    pe_n[n][:], c8_t[j][:, s, ms:me],
                                    a8_t[j][:, s, n * 512:(n + 1) * 512],
                                    start=(j == 0 and s == 0),
                                    stop=(j == JCH - 1 and s == 1))

                # t1 = rowdot(E, C): mul then reduce per psum bank
                t1p = statp.tile([128, NCH], f32, tag=f"t1p{m}")
                for n in range(NCH):
                    jk = junkp.tile([128, 512], bf16, tag="jk",
                                    name=f"jk{m}_{n}")
                    nc.vector.tensor_mul(jk[:], pe_n[n][:],
                                         cm_t[m][:, n * 512:(n + 1) * 512])
                    nc.vector.tensor_reduce(t1p[:, n:n + 1], jk[:], AX.X,
                                            OP.add)
                t1s = statp.tile([128, 1], f32, tag=f"t1s{m}")
                nc.vector.tensor_reduce(t1s[:], t1p[:], AX.X, OP.add)

                # a_in = (t1 - t2) * invcnt
                t2s = statp.tile([128, 1], f32, tag=f"t2s{m}")
                nc.vector.tensor_reduce(t2s[:], sl_t[m][:, 32:48], AX.X,
                                        OP.add)
                tin = statp.tile([128, 1], f32, tag=f"tin{m}")
                nc.vector.tensor_sub(tin[:], t1s[:], t2s[:])
                nc.vector.tensor_mul(out_t[m][:, 513:514], tin[:],
                                     sl_t[m][:, 48:49])

                # a_out = (esum - t1) * invden
                tout = statp.tile([128, 1], f32, tag=f"to{m}")
                nc.vector.tensor_sub(tout[:], es_t[m][:], t1s[:])
                nc.vector.tensor_mul(out_t[m][:, 514:515], tout[:],
                                     sl_t[m][:, 49:50])

                # ex_dist (mean), ex_clr (min)
                tex = statp.tile([128, 1], f32, tag=f"tex{m}")
                nc.vector.tensor_reduce(tex[:], sl_t[m][:, 0:16], AX.X,
                                        OP.add)
                nc.vector.tensor_scalar_mul(out_t[m][:, 516:517], tex[:],
                                            1.0 / K)
                nc.vector.tensor_reduce(out_t[m][:, 517:518],
                                        sl_t[m][:, 16:32], AX.X, OP.min)

                # h_glob broadcast
                nc.scalar.activation(out_t[m][:, D:2 * D], pgb[:], ACT.Copy)

                nc.sync.dma_start(out=out_d[m], in_=out_t[m][:])
    nc.compile()
    return nc


def _get_nc():
    if "nc" not in _NC_CACHE:
        _NC_CACHE["nc"] = _build_nc()
    return _NC_CACHE["nc"]


def kernel(h, attn_rr, attn_ro, dist_to_goal, clearance, groups):
    h = np.asarray(h, dtype=np.float32)
    attn_rr = np.asarray(attn_rr, dtype=np.float32)
    attn_ro = np.asarray(attn_ro, dtype=np.float32)
    dist_to_goal = np.asarray(dist_to_goal, dtype=np.float32)
    clearance = np.asarray(clearance, dtype=np.float32)
    groups = np.asarray(groups)

    rowsum = attn_rr.sum(axis=1)
    diag = np.ascontiguousarray(np.diagonal(attn_rr))

    # shared across cores
    EDT = BF16 if _EMODE == "bf16" else F8
    a8 = np.ascontiguousarray(
        attn_rr.astype(EDT).reshape(JCH, 2, 128, N).transpose(0, 2, 1, 3))
    hr = np.concatenate(
        [h, attn_ro, (rowsum / 64.0)[:, None]], axis=1).astype(BF16)
    hr = np.ascontiguousarray(hr.reshape(KCH, 128, HRW))

    in_maps = []
    mrow = np.arange(MLOC)[:, None]
    for s in range(NCORES):
        gs = groups[s * MLOC:(s + 1) * MLOC]
        C = np.zeros((MLOC, N), dtype=np.float32)
        np.add.at(C, (mrow, gs), 1.0)
        Ct = C.T
        c8 = np.ascontiguousarray(
            Ct.astype(EDT).reshape(JCH, 2, 128, MLOC).transpose(0, 2, 1, 3))
        ctb = np.ascontiguousarray(
            Ct.astype(BF16).reshape(KCH, 128, MLOC))
        cm = np.ascontiguousarray(C.astype(BF16).reshape(MCH, 128, N))

        mult = C[mrow, gs]                     # (MLOC, K) slot multiplicity
        sumcc = (C * C).sum(axis=1)
        nuniq = (C > 0.0).sum(axis=1)
        sl = np.zeros((MLOC, SLW), dtype=np.float32)
        sl[:, 0:16] = dist_to_goal[gs]
        sl[:, 16:32] = clearance[gs]
        sl[:, 32:48] = diag[gs] * mult
        sl[:, 48] = 1.0 / np.maximum(K * K - sumcc, 1.0)
        sl[:, 49] = 1.0 / (K * (N - nuniq))
        in_maps.append({
            "a8": a8,
            "c8": c8,
            "ctb": ctb,
            "cm": cm,
            "hr": hr,
            "sl": np.ascontiguousarray(sl.reshape(MCH, 128, SLW)),
        })

    nc = _get_nc()
    _NC_CACHE["last_in_maps"] = in_maps
    res = run_bass_kernel_spmd(nc, in_maps, list(range(NCORES)))
    return np.concatenate(
        [res.results[s]["out"].reshape(MLOC, FOUT) for s in range(NCORES)],
        axis=0)


# revision 11
# speedup vs baseline: 2.4898x; 1.4473x over previous
"""Trainium2 Bass kernel for nn_GroupFeatureBuilder (segment_reduce).

Strategy: shard the M=4096 groups across 8 cores (512 groups each).
Replace all gathers with dense matmuls against a host-built multiplicity
matrix C[m, n] = (# occurrences of robot n in group m):

  E      = C @ A            (A = attn_rr, fp8 DoubleRow matmul)
  t1[m]  = <E[m], C[m]>     = sum_{i,j} A[g_i, g_j]  (DVE mul+reduce)
  t3     ~= t1              (duplicate correction dropped; ~1e-4 rel err)
  HR     = C @ [h | attn_ro | rowsumA/64]  (fp8 C x bf16 rhs)
           -> h_g, a_obs, esum
  a_in   = (t1 - t2) * invcnt     t2, invcnt, invden host-precomputed
  a_out  = (esum - t1) * invden   from group indices / small-vector gathers
  h_glob, ex_dist/ex_clr/t2 slots: host-side input stats / gathers.

All inputs are packed host-side into partition-major tensors moved by a
handful of large dma_starts (dispatch costs ~0.6us each on the HWDGE
ring), issued in consumption order so transfers pipeline FIFO at full
HBM bandwidth. A short spin of dummy matmuls at t=0 warms the PE HAM
clock gate before the real matmul stream arrives.
"""

import numpy as np
import ml_dtypes

import concourse.bass as bass
import concourse.bacc as bacc
import concourse.tile as tile
import concourse.mybir as mybir
from concourse.bass_utils import run_bass_kernel_spmd

BF16 = ml_dtypes.bfloat16
F8 = ml_dtypes.float8_e4m3

N = 2048       # robots
D = 256        # embed
M = 4096       # groups
K = 16         # group size
NOBS = 64
NCORES = 8
MLOC = M // NCORES     # 512 groups per core
MCH = MLOC // 128      # 4 m-chunks
JCH = N // 256         # 8 double-row contraction blocks
KCH = N // 128         # 16 normal contraction blocks
NCH = N // 512         # 4 column chunks of A
HRW = D + NOBS + 1     # 321 columns of the HR rhs (h | ro | rowsumA/64)
SLW = 52               # slot cols: dg16 | cg16 | ds2_16 | invcnt | invden
FOUT = 2 * D + 6       # 518 output features
WARMUP_MM = 44         # dummy matmuls to warm the PE clock gate

f32 = mybir.dt.float32
bf16 = mybir.dt.bfloat16
f8 = mybir.dt.float8e4
OP = mybir.AluOpType
AX = mybir.AxisListType
ACT = mybir.ActivationFunctionType
DR = mybir.MatmulPerfMode.DoubleRow

_NC_CACHE = {}


def _build_nc():
    nc = bacc.Bacc("TRN2", target_bir_lowering=False, debug=False,
                   num_devices=NCORES)

    c8_d = nc.declare_dram_parameter("c8", [128, JCH, 2, MLOC], f8,
                                     isOutput=False)
    hr_d = nc.declare_dram_parameter("hr", [128, KCH, HRW], bf16,
                                     isOutput=False)
    sl_d = nc.declare_dram_parameter("sl", [128, MCH, SLW], f32,
                                     isOutput=False)
    a8_d = nc.declare_dram_parameter("a8", [128, JCH, 2, N], f8,
                                     isOutput=False)
    cm_d = nc.declare_dram_parameter("cm", [128, MCH, N], f8,
                                     isOutput=False)
    hg_d = nc.declare_dram_parameter("hg", [128, D], f32, isOutput=False)
    out_d = nc.declare_dram_parameter("out", [MCH, 128, FOUT], f32,
                                      isOutput=True)

    with tile.TileContext(nc) as tc:
        with (
            tc.tile_pool(name="res", bufs=1) as res,
            tc.tile_pool(name="junk", bufs=2) as junkp,
            tc.tile_pool(name="stats", bufs=1) as statp,
            tc.tile_pool(name="psum_e", bufs=1, space="PSUM") as pe_pool,
            tc.tile_pool(name="psum_hr", bufs=2, space="PSUM") as phr_pool,
            tc.tile_pool(name="psum_w", bufs=1, space="PSUM") as pw_pool,
        ):
            # ---- resident tiles ----
            c8_t = res.tile([128, JCH, 2, MLOC], f8, tag="c8")
            hr_t = res.tile([128, KCH, HRW], bf16, tag="hr")
            sl_t = res.tile([128, MCH, SLW], f32, tag="sl")
            a8_t = res.tile([128, JCH, 2, N], f8, tag="a8")
            cm_t = res.tile([128, MCH, N], f8, tag="cm")
            hg_t = res.tile([128, D], f32, tag="hg")

            # ---- DMA dispatches, consumption order, one HWDGE ring ----
            nc.sync.dma_start(out=c8_t[:], in_=c8_d[:])
            nc.sync.dma_start(out=hr_t[:, 0:8], in_=hr_d[:, 0:8])
            nc.sync.dma_start(out=hr_t[:, 8:16], in_=hr_d[:, 8:16])
            nc.sync.dma_start(out=sl_t[:], in_=sl_d[:])
            for jj in range(0, JCH, 2):
                nc.sync.dma_start(out=a8_t[:, jj:jj + 2],
                                  in_=a8_d[:, jj:jj + 2])
            nc.sync.dma_start(out=cm_t[:], in_=cm_d[:])
            nc.sync.dma_start(out=hg_t[:], in_=hg_d[:])

            ones_b = res.tile([128, 128], bf16, tag="ones_b")
            nc.vector.memset(ones_b[:], 1.0)

            out_t = []
            es_t = []
            for m in range(MCH):
                t = res.tile([128, FOUT], f32, tag=f"out{m}", name=f"ot{m}")
                out_t.append(t)
                nc.vector.memset(t[:, 512:513], float(K) / 3.0)
                es_t.append(statp.tile([128, 1], f32, tag=f"es{m}",
                                       name=f"es{m}"))

            # ---- PE warmup spin: release the HAM clock gate early ----
            wu = pw_pool.tile([128, 128], f32, tag="wu")
            for i in range(WARMUP_MM):
                nc.tensor.matmul(wu[:], ones_b[:], ones_b[:],
                                 start=True, stop=True)

            # ---- HR matmuls: h_g, a_obs, esum ----
            for m in range(MCH):
                ms, me = m * 128, (m + 1) * 128
                phr = phr_pool.tile([128, HRW], f32, tag="phr",
                                    name=f"phr{m}")
                for k in range(KCH):
                    nc.tensor.matmul(phr[:], c8_t[:, k // 2, k % 2, ms:me],
                                     hr_t[:, k, :],
                                     start=(k == 0), stop=(k == KCH - 1))
                nc.scalar.activation(out_t[m][:, 0:D], phr[:, 0:D], ACT.Copy,
                                     scale=1.0 / K)
                aob = statp.tile([128, 1], f32, tag=f"ao{m}", name=f"ao{m}")
                nc.vector.tensor_reduce(aob[:], phr[:, D:D + NOBS], AX.X,
                                        OP.add)
                nc.vector.tensor_scalar_mul(out_t[m][:, 515:516], aob[:],
                                            1.0 / (K * NOBS))
                nc.scalar.activation(es_t[m][:], phr[:, 320:321], ACT.Copy,
                                     scale=64.0)

            # ---- E matmuls (fp8 DoubleRow) + per-chunk stats ----
            for m in range(MCH):
                ms, me = m * 128, (m + 1) * 128
                pe_n = []
                for n in range(NCH):
                    pe_n.append(pe_pool.tile([128, 512], f32, tag=f"pe{n}",
                                             name=f"pe{m}_{n}"))
                for j in range(JCH):
                    for n in range(NCH):
                        nc.tensor.matmul(
                            pe_n[n][:], c8_t[:, j, :, ms:me],
                            a8_t[:, j, :, n * 512:(n + 1) * 512],
                            start=(j == 0), stop=(j == JCH - 1),
                            perf_mode=DR)

                # t1 = rowdot(E, C): mul then reduce per psum bank
                t1p = statp.tile([128, NCH], f32, tag=f"t1p{m}",
                                 name=f"t1p{m}")
                for n in range(NCH):
                    jk = junkp.tile([128, 512], bf16, tag="jk",
                                    name=f"jk{m}_{n}")
                    nc.vector.tensor_mul(jk[:], pe_n[n][:],
                                         cm_t[:, m, n * 512:(n + 1) * 512])
                    nc.vector.tensor_reduce(t1p[:, n:n + 1], jk[:], AX.X,
                                            OP.add)
                t1s = statp.tile([128, 1], f32, tag=f"t1s{m}", name=f"t1s{m}")
                nc.vector.tensor_reduce(t1s[:], t1p[:], AX.X, OP.add)

                # a_in = (t1 - t2) * invcnt
                t2s = statp.tile([128, 1], f32, tag=f"t2s{m}", name=f"t2s{m}")
                nc.vector.tensor_reduce(t2s[:], sl_t[:, m, 32:48], AX.X,
                                        OP.add)
                tin = statp.tile([128, 1], f32, tag=f"tin{m}", name=f"tin{m}")
                nc.vector.tensor_sub(tin[:], t1s[:], t2s[:])
                nc.vector.tensor_mul(out_t[m][:, 513:514], tin[:],
                                     sl_t[:, m, 48:49])

                # a_out = (esum - t1) * invden
                tou = statp.tile([128, 1], f32, tag=f"to{m}", name=f"to{m}")
                nc.vector.tensor_sub(tou[:], es_t[m][:], t1s[:])
                nc.vector.tensor_mul(out_t[m][:, 514:515], tou[:],
                                     sl_t[:, m, 49:50])

                # ex_dist (mean), ex_clr (min)
                tex = statp.tile([128, 1], f32, tag=f"tex{m}", name=f"tex{m}")
                nc.vector.tensor_reduce(tex[:], sl_t[:, m, 0:16], AX.X,
                                        OP.add)
                nc.vector.tensor_scalar_mul(out_t[m][:, 516:517], tex[:],
                                            1.0 / K)
                nc.vector.tensor_reduce(out_t[m][:, 517:518],
                                        sl_t[:, m, 16:32], AX.X, OP.min)

                # h_glob broadcast (host-computed)
                nc.scalar.activation(out_t[m][:, D:2 * D], hg_t[:], ACT.Copy)

                nc.scalar.dma_start(out=out_d[m], in_=out_t[m][:])
    nc.compile()
    return nc


def _get_nc():
    if "nc" not in _NC_CACHE:
        _NC_CACHE["nc"] = _build_nc()
    return _NC_CACHE["nc"]


def _host_prep(h, attn_rr, attn_ro, dist_to_goal, clearance, groups):
    h = np.asarray(h, dtype=np.float32)
    attn_rr = np.asarray(attn_rr, dtype=np.float32)
    attn_ro = np.asarray(attn_ro, dtype=np.float32)
    dist_to_goal = np.asarray(dist_to_goal, dtype=np.float32)
    clearance = np.asarray(clearance, dtype=np.float32)
    groups = np.asarray(groups)

    rowsum = attn_rr.sum(axis=1)
    diag = np.ascontiguousarray(np.diagonal(attn_rr))
    hglob = h.mean(axis=0)

    # shared across cores; robot r = 256j + 128s + p -> [p, j, s, :]
    a8 = np.ascontiguousarray(
        attn_rr.astype(F8).reshape(JCH, 2, 128, N).transpose(2, 0, 1, 3))
    hr = np.concatenate(
        [h, attn_ro, (rowsum / 64.0)[:, None]], axis=1).astype(BF16)
    hr = np.ascontiguousarray(hr.reshape(KCH, 128, HRW).transpose(1, 0, 2))
    hg = np.ascontiguousarray(
        np.broadcast_to(hglob[None, :], (128, D)).astype(np.float32))

    in_maps = []
    mrow = np.arange(MLOC)[:, None]
    for s in range(NCORES):
        gs = groups[s * MLOC:(s + 1) * MLOC]
        C = np.zeros((MLOC, N), dtype=np.float32)
        np.add.at(C, (mrow, gs), 1.0)
        c8 = np.ascontiguousarray(C.T.astype(F8).reshape(
            JCH, 2, 128, MLOC).transpose(2, 0, 1, 3))
        cm = np.ascontiguousarray(
            C.astype(F8).reshape(MCH, 128, N).transpose(1, 0, 2))

        mult = C[mrow, gs]                     # (MLOC, K) slot multiplicity
        sumcc = (C * C).sum(axis=1)
        nuniq = (C > 0.0).sum(axis=1)
        sl = np.zeros((MLOC, SLW), dtype=np.float32)
        sl[:, 0:16] = dist_to_goal[gs]
        sl[:, 16:32] = clearance[gs]
        sl[:, 32:48] = diag[gs] * mult
        sl[:, 48] = 1.0 / np.maximum(K * K - sumcc, 1.0)
        sl[:, 49] = 1.0 / (K * (N - nuniq))
        sl = np.ascontiguousarray(
            sl.reshape(MCH, 128, SLW).transpose(1, 0, 2))
        in_maps.append({
            "a8": a8, "c8": c8, "cm": cm, "hr": hr, "sl": sl, "hg": hg,
        })
    return in_maps


def kernel(h, attn_rr, attn_ro, dist_to_goal, clearance, groups):
    in_maps = _host_prep(h, attn_rr, attn_ro, dist_to_goal, clearance, groups)
    nc = _get_nc()
    _NC_CACHE["last_in_maps"] = in_maps
    res = run_bass_kernel_spmd(nc, in_maps, list(range(NCORES)))
    return np.concatenate(
        [res.results[s]["out"].reshape(MLOC, FOUT) for s in range(NCORES)],
        axis=0)
